# revision 2
# baseline (speedup 1.0000x reference)
"""Trainium2 Bass kernel for DCRNN-Temporal (gnn_message_passing), v4.

Contract: kernel(**inputs) takes FULL numpy inputs and returns the FULL
[N, 12] output, running a Bass SPMD kernel on 8 NeuronCores.

Math (H0 = 0 simplifies the DCRNN cell; R unused):
  T1o = P_f x, T1i = P_r x, Y2o = P_f T1o, Y2i = P_r T1i
  G   = x@WXeff + T1o@w01 + T1i@w11 + Y2o@(2 w02) + Y2i@(2 w12) + b
  H   = sigmoid(-Gz - bz) * tanh(Gh + bh)
  out = relu(H) @ w_lin + b_lin

Implementation: nodes partitioned by destination across 8 cores.  All
per-edge gathers run as gpsimd ap_gather (Q7 ucode, SBUF->SBUF) from
fp16 feature-pair-major window tables resident in SBUF:

  8 slabs of 16 partitions; slab k = (copy k//4, window (k//2)%2,
  dir k%2).  Table row 16k+p, col 2i+h = scaled_x[window_base+i,
  feature 2p+h].  Each (window, dir) edge set is split 50/50 between its
  two copy slabs, so every edge is gathered by exactly one Q7 core.

  Accumulation is scatter-free ELL into an fp16 accumulator: per
  (core, dir) the dsts are ranked by padded count m(v) =
  max(ceil(cA/2), ceil(cB/2)); round sizes NJ(j) are maxed across all
  (core, dir) so one [128, 2*NJ] DVE add per round covers all 8 slabs.

  Slab partials reduce via a TensorE selector matmul (quadruplicated
  rows) and scale by 1/deg; the AllGather ships the table in RANK order
  (hop-2 idx lists are composed with each source core's rank
  permutation on the host), so the natural-order permute (ap_gather)
  only feeds the gate-side T1 recovery and runs concurrently with the
  collective.  Hop-2 windows load back as 4 wide 32-partition DMAs,
  then the second hop runs with the swapped idx lists, followed by
  fp16 gate matmuls (strided pair-major rhs) + activations + head.
"""

import os
import sys

for _p in ("/opt/trn_rl_repo", "/root/.axon_site/_ro/trn_rl_repo"):
    if os.path.isdir(_p) and _p not in sys.path:
        sys.path.insert(0, _p)
        break

import numpy as np

import concourse.bass as bass
import concourse.tile as tile
from concourse import bacc, mybir
from concourse import bass_utils

F16NP = np.float16

N = 50000
P = 8
NLOC = 6250
F = 32
FO = 64
PER = 12
HALF = 25000
NE = 25002          # window table node slots (25000 + 2 zero cols)
PADIDX = 25000      # zero column
HP = 6272           # rank/natural node space width (NLOC padded)
PQ = 1664           # max quarter length (quarters 1536,1536,1536,1664)
QS = (0, 1536, 3072, 4608)
QL = (1536, 1536, 1536, 1664)
NCALLS = 8          # ap_gather calls per hop (CHUNK sized from EH)
NT = 512            # gate node chunk

FP32 = mybir.dt.float32
FP16 = mybir.dt.float16
I16 = mybir.dt.int16


def _ceil(a, b):
    return -(-a // b)


def _wrap16(lst):
    """[n] -> [16, n//16] int16 (idx j at partition j%16, col j//16)."""
    return np.ascontiguousarray(lst.reshape(-1, 16).T.astype(np.int16))


def _slab(sub, w, d):
    """slab index: copy*4 + window*2 + dir."""
    return sub * 4 + w * 2 + d


# ----------------------------------------------------------------------------
# Host-side preprocessing
# ----------------------------------------------------------------------------

def preprocess(x, edge_index, edge_weight, w_z, b_z, w_r, b_r, w_h, b_h,
               w_lin, b_lin):
    x = np.asarray(x, np.float32)
    row = np.asarray(edge_index[0], np.int64)
    col = np.asarray(edge_index[1], np.int64)
    ew = np.asarray(edge_weight, np.float64)

    deg_out = np.bincount(row, weights=ew, minlength=N)
    deg_in = np.bincount(col, weights=ew, minlength=N)
    doi = np.where(deg_out > 0, 1.0 / np.maximum(deg_out, 1e-300), 0.0)
    dii = np.where(deg_in > 0, 1.0 / np.maximum(deg_in, 1e-300), 0.0)
    doi1 = np.where(deg_out > 0, doi, 1.0).astype(np.float32)
    dii1 = np.where(deg_in > 0, dii, 1.0).astype(np.float32)
    dgo1 = np.where(deg_out > 0, deg_out, 1.0).astype(np.float32)
    dgi1 = np.where(deg_in > 0, deg_in, 1.0).astype(np.float32)
    doi = doi.astype(np.float32)
    dii = dii.astype(np.float32)

    # hop-1 table (same for all cores): [128, 2*NE] fp16
    xf = x * doi[:, None]
    xr = x * dii[:, None]

    def pack(S, w):  # [N, 32] window w -> [16, 2*NE]
        seg = S[w * HALF:(w + 1) * HALF]
        blk = np.zeros((16, 2 * NE), np.float32)
        blk[:, :2 * HALF] = seg.reshape(HALF, 16, 2).transpose(1, 0, 2)\
            .reshape(16, 2 * HALF)
        return blk

    blocks = []
    for k in range(8):
        d = k % 2
        w = (k // 2) % 2
        blocks.append(pack(xr if d else xf, w))
    tab1 = np.ascontiguousarray(np.concatenate(blocks, 0).astype(F16NP))

    # --- edge streams: per (core, dir) ELL structures ---------------------
    percd = []  # index d*P + c
    for d, (src, dst) in enumerate(((row, col), (col, row))):
        core = dst // NLOC
        for c in range(P):
            sel = np.nonzero(core == c)[0]
            s = src[sel]
            t = dst[sel] - c * NLOC
            w = (s >= HALF).astype(np.int64)
            order = np.lexsort((s, t, w))
            sw, tw, ww = s[order], t[order], w[order]
            grp = ww * NLOC + tw
            cnts = np.bincount(grp, minlength=2 * NLOC)
            ptr = np.zeros(2 * NLOC + 1, np.int64)
            np.cumsum(cnts, out=ptr[1:])
            occ = np.arange(len(order), dtype=np.int64) - ptr[grp]
            sub = occ % 2
            j = occ // 2
            cA, cB = cnts[:NLOC], cnts[NLOC:]
            m = np.maximum((cA + 1) // 2, (cB + 1) // 2)
            percd.append(dict(c=c, d=d, sw=sw, tw=tw, ww=ww, sub=sub, j=j,
                              m=m))

    # global common round sizes
    mxr = max(int(e["m"].max()) for e in percd)
    NJ = np.zeros(mxr, np.int64)
    for e in percd:
        h = np.bincount(e["m"], minlength=mxr + 1)
        ncd = NLOC - np.cumsum(h)[:-1]
        NJ = np.maximum(NJ, ncd)
    off = np.zeros(mxr + 1, np.int64)
    np.cumsum(NJ, out=off[1:])
    EH = int(off[-1])
    # chunk % 32 keeps each idx slice 4B-aligned per partition (the Q7
    # ucode pops idx pairs as 4B words)
    chunk = _ceil(EH, NCALLS * 32) * 32
    EHP = NCALLS * chunk

    for e in percd:
        ordm = np.argsort(-e["m"], kind="stable")
        rank = np.empty(NLOC, np.int64)
        rank[ordm] = np.arange(NLOC)
        e["rank"] = rank
        e["ordm"] = ordm

    # effective gate weights
    def eff(wg):
        wg = np.asarray(wg, np.float32)
        wxe = wg[0, 0, :F] + wg[1, 0, :F] - wg[0, 2, :F] - wg[1, 2, :F]
        return wxe, wg[0, 1, :F], wg[1, 1, :F], 2 * wg[0, 2, :F], \
            2 * wg[1, 2, :F]

    wxz, w1oz, w1iz, w2oz, w2iz = eff(w_z)
    wxh, w1oh, w1ih, w2oh, w2ih = eff(w_h)

    WX = np.ascontiguousarray(
        np.concatenate([wxz, wxh], axis=1).astype(F16NP))

    # WQ [128, 16*128] fp16: blocks (term, h, q): term 0=Y2, 1=T1
    WQ = np.zeros((128, 16 * 128), np.float32)
    for term, ((woz_, woh_), (wiz_, wih_)) in enumerate(
            (((w2oz, w2oh), (w2iz, w2ih)), ((w1oz, w1oh), (w1iz, w1ih)))):
        for h in range(2):
            for q in range(4):
                b = (term * 2 + h) * 4 + q
                blk = np.zeros((128, 128), np.float32)
                for mm in range(16):
                    blk[32 * q + mm, 0:FO] = woz_[2 * mm + h]
                    blk[32 * q + mm, FO:128] = woh_[2 * mm + h]
                    blk[32 * q + 16 + mm, 0:FO] = wiz_[2 * mm + h]
                    blk[32 * q + 16 + mm, FO:128] = wih_[2 * mm + h]
                WQ[:, 128 * b:128 * (b + 1)] = blk
    WQ = np.ascontiguousarray(WQ.astype(F16NP))

    # selector [128, 128] f32: slab k (dir k%2) pair pp -> rows 32q+16*dir+pp
    selw = np.zeros((128, 128), np.float32)
    for k in range(8):
        dirk = k % 2
        for pp in range(16):
            for q in range(4):
                selw[16 * k + pp, 32 * q + 16 * dirk + pp] = 1.0

    nbz = np.ascontiguousarray(-np.asarray(b_z, np.float32)[:, None])
    bhv = np.ascontiguousarray(np.asarray(b_h, np.float32)[:, None])
    WL = np.ascontiguousarray(np.asarray(w_lin, np.float32))
    BL = np.ascontiguousarray(np.asarray(b_lin, np.float32)[:, None])

    in_maps = []
    for c in range(P):
        ef = percd[c]
        er = percd[P + c]

        # gather idx [128, EHP//16], hop-1 (natural table cols) and hop-2
        # (rank-composed cols of the AllGather'ed tables)
        gidx = np.empty((128, EHP // 16), np.int16)
        gidx2 = np.empty((128, EHP // 16), np.int16)
        for d, e in ((0, ef), (1, er)):
            slots = off[e["j"]] + e["rank"][e["tw"]]
            vals1 = e["sw"] - e["ww"] * HALF
            cs = e["sw"] // NLOC
            ls = e["sw"] - cs * NLOC
            rk = np.empty(len(cs), np.int64)
            for csv in range(P):
                msk = cs == csv
                rk[msk] = percd[d * P + csv]["rank"][ls[msk]]
            vals2 = (cs % 4) * NLOC + rk
            for sub in range(2):
                for wv in range(2):
                    msk = (e["sub"] == sub) & (e["ww"] == wv)
                    k = _slab(sub, wv, d)
                    iv = np.full(EHP, PADIDX, np.int64)
                    iv[slots[msk]] = vals1[msk]
                    gidx[16 * k:16 * k + 16] = _wrap16(iv)
                    iv2 = np.full(EHP, PADIDX, np.int64)
                    iv2[slots[msk]] = vals2[msk]
                    gidx2[16 * k:16 * k + 16] = _wrap16(iv2)

        # perm idx [128, PQ//16] (core 2q+dd handles quarter q of dir dd)
        pidx = np.empty((128, PQ // 16), np.int16)
        for q in range(4):
            for dd, e in ((0, ef), (1, er)):
                L = np.full(PQ, HP - 1, np.int64)
                qn = min(QL[q], NLOC - QS[q])
                nat = np.arange(QS[q], QS[q] + qn)
                L[:qn] = e["rank"][nat]
                k = 2 * q + dd
                pidx[16 * k:16 * k + 16] = _wrap16(L)

        # dscale [128, 2*HP] fp16 (rank order, quadruplicated)
        dsc = np.zeros((32, HP), np.float32)
        gl = c * NLOC
        dsc[0:16, :NLOC] = doi1[gl + ef["ordm"]][None, :]
        dsc[16:32, :NLOC] = dii1[gl + er["ordm"]][None, :]
        dsc = np.repeat(dsc[:, :, None], 2, axis=2).reshape(32, 2 * HP)
        dsc = np.tile(dsc, (4, 1)).astype(F16NP)

        # recovery [128, 2*PQ] f32 (natural order, quarter layout)
        rcv = np.zeros((128, 2 * PQ), np.float32)
        for q in range(4):
            qn = min(QL[q], NLOC - QS[q])
            nat = gl + QS[q] + np.arange(qn)
            rcv[32 * q:32 * q + 16, :2 * qn] = np.repeat(dgo1[nat], 2)[None]
            rcv[32 * q + 16:32 * q + 32, :2 * qn] = \
                np.repeat(dgi1[nat], 2)[None]

        # xT [32, HP] fp16 natural feature-major
        xT = np.zeros((F, HP), np.float32)
        xT[:, :NLOC] = x[gl:gl + NLOC].T
        xT = xT.astype(F16NP)

        in_maps.append({
            "tab1": tab1, "gidx": gidx, "gidx2": gidx2, "pidx": pidx,
            "dsc": np.ascontiguousarray(dsc),
            "rcv": np.ascontiguousarray(rcv),
            "xT": np.ascontiguousarray(xT),
            "selw": selw.astype(F16NP), "wq": WQ, "wx": WX,
            "nbz": nbz, "bh": bhv, "wl": WL, "bl": BL,
        })

    meta = dict(NJ=[int(v) for v in NJ], off=[int(v) for v in off],
                EH=EH, EHP=EHP, chunk=chunk)
    return in_maps, meta


# ----------------------------------------------------------------------------
# Device program
# ----------------------------------------------------------------------------

def build_program(meta):
    NJ, off, EHP = meta["NJ"], meta["off"], meta["EHP"]
    CHUNK = meta["chunk"]
    nrounds = len(NJ)
    NSEL = _ceil(2 * HP, 512)
    gate_chunks = []
    n0 = 0
    while n0 < HP:
        n1 = min(n0 + NT, HP)
        q = 3
        for qq in range(4):
            if n0 < QS[qq] + QL[qq]:
                q = qq
                break
        gate_chunks.append((n0, n1, q))
        n0 = n1

    nc = bacc.Bacc("TRN2", target_bir_lowering=False, debug=False,
                   num_devices=P)

    def din(name, shape, dt=FP32):
        return nc.dram_tensor(name, list(shape), dt, kind="ExternalInput").ap()

    tab1_d = din("tab1", (128, 2 * NE), FP16)
    gidx_d = din("gidx", (128, EHP // 16), I16)
    gidx2_d = din("gidx2", (128, EHP // 16), I16)
    pidx_d = din("pidx", (128, PQ // 16), I16)
    dsc_d = din("dsc", (128, 2 * HP), FP16)
    rcv_d = din("rcv", (128, 2 * PQ), FP32)
    xT_d = din("xT", (F, HP), FP16)
    selw_d = din("selw", (128, 128), FP16)
    wq_d = din("wq", (128, 16 * 128), FP16)
    wx_d = din("wx", (F, 128), FP16)
    nbz_d = din("nbz", (FO, 1), FP32)
    bh_d = din("bh", (FO, 1), FP32)
    wl_d = din("wl", (FO, PER), FP32)
    bl_d = din("bl", (PER, 1), FP32)
    out_d = nc.dram_tensor("out", [PER, HP], FP32, kind="ExternalOutput").ap()

    with tile.TileContext(nc) as tc:
        from contextlib import ExitStack
        with ExitStack() as ctx:
            sb = ctx.enter_context(tc.tile_pool(name="sb", bufs=1))
            gp = ctx.enter_context(tc.tile_pool(name="gp", bufs=2))
            dp = ctx.enter_context(tc.tile_pool(name="dp", bufs=2))
            xp = ctx.enter_context(tc.tile_pool(name="xp", bufs=2))
            op = ctx.enter_context(tc.tile_pool(name="op", bufs=2))
            ps = ctx.enter_context(tc.tile_pool(name="ps", bufs=2,
                                                space="PSUM"))
            pg = ctx.enter_context(tc.tile_pool(name="pg", bufs=2,
                                                space="PSUM"))
            ph = ctx.enter_context(tc.tile_pool(name="ph", bufs=2,
                                                space="PSUM"))
            dr = ctx.enter_context(tc.tile_pool(name="dr", bufs=1,
                                                space="DRAM"))

            tab = sb.tile([128, 2 * NE], FP16, tag="tab")
            gidx = sb.tile([128, EHP // 16], I16, tag="gidx")
            pidx = sb.tile([128, PQ // 16], I16, tag="pidx")
            rcv = sb.tile([128, 2 * PQ], FP32, tag="rcv")
            selw = sb.tile([128, 128], FP16, tag="selw")
            wq = sb.tile([128, 16 * 128], FP16, tag="wq")
            wx = sb.tile([F, 128], FP16, tag="wx")
            nbz = sb.tile([FO, 1], FP32, tag="nbz")
            bh = sb.tile([FO, 1], FP32, tag="bh")
            wl = sb.tile([FO, PER], FP32, tag="wl")
            bl = sb.tile([PER, 1], FP32, tag="bl")
            acc = sb.tile([128, 2 * HP], FP16, tag="acc")
            t1n = sb.tile([128, 2 * PQ], FP16, tag="t1n")

            cT = dr.tile([32, 2 * NLOC], FP16, tag="cT")
            agT = dr.tile([256, 2 * NLOC], FP16, tag="agT")

            nc.sync.dma_start(tab[:], tab1_d)
            nc.sync.dma_start(gidx[:], gidx_d)
            nc.sync.dma_start(pidx[:], pidx_d)
            nc.sync.dma_start(rcv[:], rcv_d)
            nc.sync.dma_start(selw[:], selw_d)
            nc.sync.dma_start(wq[:], wq_d)
            nc.sync.dma_start(wx[:], wx_d)
            nc.sync.dma_start(nbz[:], nbz_d)
            nc.sync.dma_start(bh[:], bh_d)
            nc.sync.dma_start(wl[:], wl_d)
            nc.sync.dma_start(bl[:], bl_d)
            nc.vector.memset(acc[:], 0.0)

            tabv = tab[:].rearrange("p (n d) -> p n d", d=2)

            def hop():
                for cc in range(NCALLS):
                    g = gp.tile([128, CHUNK, 2], FP16, tag="g")
                    nc.gpsimd.ap_gather(
                        out_ap=g[:],
                        in_ap=tabv,
                        idxs_ap=gidx[:, cc * (CHUNK // 16):
                                     (cc + 1) * (CHUNK // 16)],
                        channels=128,
                        num_elems=NE,
                        d=2,
                        num_idxs=CHUNK,
                    )
                    gv = g[:].rearrange("p n d -> p (n d)")
                    c0, c1 = cc * CHUNK, (cc + 1) * CHUNK
                    for j in range(nrounds):
                        s = max(off[j], c0)
                        e = min(off[j] + NJ[j], c1)
                        if s >= e:
                            continue
                        r0 = s - off[j]
                        nc.vector.tensor_tensor(
                            out=acc[:, 2 * r0:2 * (r0 + e - s)],
                            in0=acc[:, 2 * r0:2 * (r0 + e - s)],
                            in1=gv[:, 2 * (s - c0):2 * (e - c0)],
                            op=mybir.AluOpType.add,
                        )

            def selpass(scale):
                """sel-matmul acc -> fp16 (scaled if scale) into tab[:,
                0:2*HP] (tab is dead at both call sites)."""
                accb = tab[:, 0:2 * HP]
                for si in reversed(range(NSEL)):
                    s0 = si * 512
                    s1 = min(s0 + 512, 2 * HP)
                    pt = ps.tile([128, 512], FP32, tag="sel")
                    nc.tensor.matmul(out=pt[:, :s1 - s0], lhsT=selw[:],
                                     rhs=acc[:, s0:s1], start=True, stop=True)
                    if scale:
                        dt_ = dp.tile([128, 512], FP16, tag="dsc")
                        nc.sync.dma_start(dt_[:, :s1 - s0], dsc_d[:, s0:s1])
                        nc.vector.tensor_tensor(
                            out=accb[:, s0:s1], in0=pt[:, :s1 - s0],
                            in1=dt_[:, :s1 - s0], op=mybir.AluOpType.mult)
                    else:
                        nc.vector.tensor_copy(out=accb[:, s0:s1],
                                              in_=pt[:, :s1 - s0])
                return accb

            def permute(src, dst):
                nc.gpsimd.ap_gather(
                    out_ap=dst.rearrange("p (n d) -> p n d", d=2),
                    in_ap=src.rearrange("p (n d) -> p n d", d=2),
                    idxs_ap=pidx[:],
                    channels=128,
                    num_elems=HP,
                    d=2,
                    num_idxs=PQ,
                )

            # ---- hop 1 -----------------------------------------------------
            hop()
            accb1 = selpass(scale=True)
            # re-zero acc early (overlaps collective/loads)
            nc.vector.memset(acc[:], 0.0)
            # contribution in RANK order straight from the sel output
            nc.scalar.dma_start(cT[:], accb1[0:32, 0:2 * NLOC])
            nc.gpsimd.collective_compute(
                "AllGather", mybir.AluOpType.bypass,
                replica_groups=[list(range(P))],
                ins=[cT[:].opt()],
                outs=[agT[:].opt()],
            )
            # swap in hop-2 idx lists (overwrites gidx after hop-1 reads)
            nc.sync.dma_start(gidx[:], gidx2_d)
            # natural-order T1 for the gates: permute + recovery, runs on
            # Q7/DVE concurrently with the collective
            pmo = tab[:, 2 * HP:2 * HP + 2 * PQ]
            permute(accb1, pmo)
            nc.vector.tensor_tensor(out=t1n[:], in0=pmo[:, :], in1=rcv[:],
                                    op=mybir.AluOpType.mult)

            # ---- hop-2 table load: 4 wide 32-partition DMAs ---------------
            agv = agT[:].rearrange("(w c2 r) c -> w r c2 c", w=2, c2=4, r=32)
            for cp, w in ((0, 0), (0, 1), (1, 0), (1, 1)):
                eng = nc.sync if cp == w else nc.scalar
                p0 = 64 * cp + 32 * w
                eng.dma_start(
                    tab[p0:p0 + 32, 0:2 * 4 * NLOC]
                    .rearrange("p (c2 c) -> p c2 c", c2=4),
                    agv[w])

            # ---- hop 2 -----------------------------------------------------
            hop()
            accb2 = selpass(scale=False)
            y2n = tab[:, 2 * HP:2 * HP + 2 * PQ]
            permute(accb2, y2n)

            # ---- gates + head ---------------------------------------------
            t1v = t1n[:].rearrange("p (n d) -> p n d", d=2)
            y2v = y2n.rearrange("p (n d) -> p n d", d=2)
            for (n0, n1, q) in gate_chunks:
                n = n1 - n0
                lc = n0 - QS[q]
                xt = xp.tile([F, NT], FP16, tag="xt")
                nc.sync.dma_start(xt[:, :n], xT_d[:, n0:n1])
                pz = pg.tile([FO, NT], FP32, tag="pz")
                phh = pg.tile([FO, NT], FP32, tag="ph")
                for half, pt in ((0, pz), (1, phh)):
                    co = FO * half
                    first = True
                    for term, vv in ((0, y2v), (1, t1v)):
                        for hh in range(2):
                            b = (term * 2 + hh) * 4 + q
                            nc.tensor.matmul(
                                out=pt[:, :n],
                                lhsT=wq[:, 128 * b + co:128 * b + co + FO],
                                rhs=vv[:, lc:lc + n, hh],
                                start=first, stop=False)
                            first = False
                    nc.tensor.matmul(out=pt[:, :n],
                                     lhsT=wx[:, co:co + FO],
                                     rhs=xt[:, :n], start=False, stop=True)
                AF = mybir.ActivationFunctionType
                gz = op.tile([FO, NT], FP32, tag="gz")
                gh = op.tile([FO, NT], FP32, tag="gh")
                nc.scalar.activation(out=gz[:, :n], in_=pz[:, :n],
                                     func=AF.Sigmoid, bias=nbz[:], scale=-1.0)
                nc.scalar.activation(out=gh[:, :n], in_=phh[:, :n],
                                     func=AF.Tanh, bias=bh[:], scale=1.0)
                nc.vector.tensor_tensor(out=gz[:, :n], in0=gz[:, :n],
                                        in1=gh[:, :n],
                                        op=mybir.AluOpType.mult)
                nc.vector.tensor_scalar_max(gz[:, :n], gz[:, :n], 0.0)
                po = ph.tile([PER, NT], FP32, tag="po")
                nc.tensor.matmul(out=po[:, :n], lhsT=wl[:], rhs=gz[:, :n],
                                 start=True, stop=True)
                ot = op.tile([PER, NT], FP32, tag="ot")
                nc.scalar.add(out=ot[:, :n], in_=po[:, :n], add=bl[:])
                nc.scalar.dma_start(out_d[:, n0:n1], ot[:, :n])

    nc.compile()
    return nc


# ----------------------------------------------------------------------------
# Entry point
# ----------------------------------------------------------------------------

def kernel(x, edge_index, edge_weight, w_z, b_z, w_r, b_r, w_h, b_h,
           w_lin, b_lin, _trace=False):
    in_maps, meta = preprocess(x, edge_index, edge_weight, w_z, b_z, w_r,
                               b_r, w_h, b_h, w_lin, b_lin)
    nc = build_program(meta)
    res = bass_utils.run_bass_kernel_spmd(
        nc, in_maps, core_ids=list(range(P)), trace=_trace)
    out = np.empty((N, PER), np.float32)
    for c in range(P):
        out[c * NLOC:(c + 1) * NLOC] = res.results[c]["out"].T[:NLOC]
    if _trace:
        return out, res
    return out


# revision 3
# speedup vs baseline: 1.0080x; 1.0080x over previous
"""Trainium2 Bass kernel for DCRNN-Temporal (gnn_message_passing), v4.

Contract: kernel(**inputs) takes FULL numpy inputs and returns the FULL
[N, 12] output, running a Bass SPMD kernel on 8 NeuronCores.

Math (H0 = 0 simplifies the DCRNN cell; R unused):
  T1o = P_f x, T1i = P_r x, Y2o = P_f T1o, Y2i = P_r T1i
  G   = x@WXeff + T1o@w01 + T1i@w11 + Y2o@(2 w02) + Y2i@(2 w12) + b
  H   = sigmoid(-Gz - bz) * tanh(Gh + bh)
  out = relu(H) @ w_lin + b_lin

Implementation: nodes partitioned by destination across 8 cores.  All
per-edge gathers run as gpsimd ap_gather (Q7 ucode, SBUF->SBUF) from
fp16 feature-pair-major window tables resident in SBUF:

  8 slabs of 16 partitions; slab k = (copy k//4, window (k//2)%2,
  dir k%2).  Table row 16k+p, col 2i+h = scaled_x[window_base+i,
  feature 2p+h].  Each (window, dir) edge set is split 50/50 between its
  two copy slabs, so every edge is gathered by exactly one Q7 core.

  Accumulation is scatter-free ELL into an fp16 accumulator: per
  (core, dir) the dsts are ranked by padded count m(v) =
  max(ceil(cA/2), ceil(cB/2)); round sizes NJ(j) are maxed across all
  (core, dir) so one [128, 2*NJ] DVE add per round covers all 8 slabs.

  Slab partials reduce via a TensorE selector matmul (quadruplicated
  rows) and scale by 1/deg; the AllGather ships the table in RANK order
  (hop-2 idx lists are composed with each source core's rank
  permutation on the host), so the natural-order permute (ap_gather)
  only feeds the gate-side T1 recovery and runs concurrently with the
  collective.  Hop-2 windows load back as 4 wide 32-partition DMAs,
  then the second hop runs with the swapped idx lists, followed by
  fp16 gate matmuls (strided pair-major rhs) + activations + head.
"""

import os
import sys

for _p in ("/opt/trn_rl_repo", "/root/.axon_site/_ro/trn_rl_repo"):
    if os.path.isdir(_p) and _p not in sys.path:
        sys.path.insert(0, _p)
        break

import numpy as np

import concourse.bass as bass
import concourse.tile as tile
from concourse import bacc, mybir
from concourse import bass_utils

F16NP = np.float16

N = 50000
P = 8
NLOC = 6250
F = 32
FO = 64
PER = 12
HALF = 25000
NE = 25002          # window table node slots (25000 + 2 zero cols)
PADIDX = 25000      # zero column
HP = 6272           # rank/natural node space width (NLOC padded)
PQ = 1664           # max quarter length (quarters 1536,1536,1536,1664)
QS = (0, 1536, 3072, 4608)
QL = (1536, 1536, 1536, 1664)
NCALLS = 8          # ap_gather calls per hop (CHUNK sized from EH)
NT = 512            # gate node chunk

FP32 = mybir.dt.float32
FP16 = mybir.dt.float16
I16 = mybir.dt.int16


def _ceil(a, b):
    return -(-a // b)


def _wrap16(lst):
    """[n] -> [16, n//16] int16 (idx j at partition j%16, col j//16)."""
    return np.ascontiguousarray(lst.reshape(-1, 16).T.astype(np.int16))


def _slab(sub, w, d):
    """slab index: copy*4 + window*2 + dir."""
    return sub * 4 + w * 2 + d


# ----------------------------------------------------------------------------
# Host-side preprocessing
# ----------------------------------------------------------------------------

def preprocess(x, edge_index, edge_weight, w_z, b_z, w_r, b_r, w_h, b_h,
               w_lin, b_lin):
    x = np.asarray(x, np.float32)
    row = np.asarray(edge_index[0], np.int64)
    col = np.asarray(edge_index[1], np.int64)
    ew = np.asarray(edge_weight, np.float64)

    deg_out = np.bincount(row, weights=ew, minlength=N)
    deg_in = np.bincount(col, weights=ew, minlength=N)
    doi = np.where(deg_out > 0, 1.0 / np.maximum(deg_out, 1e-300), 0.0)
    dii = np.where(deg_in > 0, 1.0 / np.maximum(deg_in, 1e-300), 0.0)
    doi1 = np.where(deg_out > 0, doi, 1.0).astype(np.float32)
    dii1 = np.where(deg_in > 0, dii, 1.0).astype(np.float32)
    dgo1 = np.where(deg_out > 0, deg_out, 1.0).astype(np.float32)
    dgi1 = np.where(deg_in > 0, deg_in, 1.0).astype(np.float32)
    doi = doi.astype(np.float32)
    dii = dii.astype(np.float32)

    # hop-1 table (same for all cores): [128, 2*NE] fp16
    xf = x * doi[:, None]
    xr = x * dii[:, None]

    def pack(S, w):  # [N, 32] window w -> [16, 2*NE]
        seg = S[w * HALF:(w + 1) * HALF]
        blk = np.zeros((16, 2 * NE), np.float32)
        blk[:, :2 * HALF] = seg.reshape(HALF, 16, 2).transpose(1, 0, 2)\
            .reshape(16, 2 * HALF)
        return blk

    blocks = []
    for k in range(8):
        d = k % 2
        w = (k // 2) % 2
        blocks.append(pack(xr if d else xf, w))
    tab1 = np.ascontiguousarray(np.concatenate(blocks, 0).astype(F16NP))

    # --- edge streams: per (core, dir) ELL structures ---------------------
    percd = []  # index d*P + c
    for d, (src, dst) in enumerate(((row, col), (col, row))):
        core = dst // NLOC
        for c in range(P):
            sel = np.nonzero(core == c)[0]
            s = src[sel]
            t = dst[sel] - c * NLOC
            w = (s >= HALF).astype(np.int64)
            order = np.lexsort((s, t, w))
            sw, tw, ww = s[order], t[order], w[order]
            grp = ww * NLOC + tw
            cnts = np.bincount(grp, minlength=2 * NLOC)
            ptr = np.zeros(2 * NLOC + 1, np.int64)
            np.cumsum(cnts, out=ptr[1:])
            occ = np.arange(len(order), dtype=np.int64) - ptr[grp]
            sub = occ % 2
            j = occ // 2
            cA, cB = cnts[:NLOC], cnts[NLOC:]
            m = np.maximum((cA + 1) // 2, (cB + 1) // 2)
            percd.append(dict(c=c, d=d, sw=sw, tw=tw, ww=ww, sub=sub, j=j,
                              m=m))

    # global common round sizes
    mxr = max(int(e["m"].max()) for e in percd)
    NJ = np.zeros(mxr, np.int64)
    for e in percd:
        h = np.bincount(e["m"], minlength=mxr + 1)
        ncd = NLOC - np.cumsum(h)[:-1]
        NJ = np.maximum(NJ, ncd)
    off = np.zeros(mxr + 1, np.int64)
    np.cumsum(NJ, out=off[1:])
    EH = int(off[-1])
    # chunk % 32 keeps each idx slice 4B-aligned per partition (the Q7
    # ucode pops idx pairs as 4B words)
    chunk = _ceil(EH, NCALLS * 32) * 32
    EHP = NCALLS * chunk

    for e in percd:
        ordm = np.argsort(-e["m"], kind="stable")
        rank = np.empty(NLOC, np.int64)
        rank[ordm] = np.arange(NLOC)
        e["rank"] = rank
        e["ordm"] = ordm

    # effective gate weights
    def eff(wg):
        wg = np.asarray(wg, np.float32)
        wxe = wg[0, 0, :F] + wg[1, 0, :F] - wg[0, 2, :F] - wg[1, 2, :F]
        return wxe, wg[0, 1, :F], wg[1, 1, :F], 2 * wg[0, 2, :F], \
            2 * wg[1, 2, :F]

    wxz, w1oz, w1iz, w2oz, w2iz = eff(w_z)
    wxh, w1oh, w1ih, w2oh, w2ih = eff(w_h)

    WX = np.ascontiguousarray(
        np.concatenate([wxz, wxh], axis=1).astype(F16NP))

    # WQ [128, 16*128] fp16: blocks (term, h, q): term 0=Y2, 1=T1
    WQ = np.zeros((128, 16 * 128), np.float32)
    for term, ((woz_, woh_), (wiz_, wih_)) in enumerate(
            (((w2oz, w2oh), (w2iz, w2ih)), ((w1oz, w1oh), (w1iz, w1ih)))):
        for h in range(2):
            for q in range(4):
                b = (term * 2 + h) * 4 + q
                blk = np.zeros((128, 128), np.float32)
                for mm in range(16):
                    blk[32 * q + mm, 0:FO] = woz_[2 * mm + h]
                    blk[32 * q + mm, FO:128] = woh_[2 * mm + h]
                    blk[32 * q + 16 + mm, 0:FO] = wiz_[2 * mm + h]
                    blk[32 * q + 16 + mm, FO:128] = wih_[2 * mm + h]
                WQ[:, 128 * b:128 * (b + 1)] = blk
    WQ = np.ascontiguousarray(WQ.astype(F16NP))

    # selector [128, 128] f32: slab k (dir k%2) pair pp -> rows 32q+16*dir+pp
    selw = np.zeros((128, 128), np.float32)
    for k in range(8):
        dirk = k % 2
        for pp in range(16):
            for q in range(4):
                selw[16 * k + pp, 32 * q + 16 * dirk + pp] = 1.0

    nbz = np.ascontiguousarray(-np.asarray(b_z, np.float32)[:, None])
    bhv = np.ascontiguousarray(np.asarray(b_h, np.float32)[:, None])
    WL = np.ascontiguousarray(np.asarray(w_lin, np.float32))
    BL = np.ascontiguousarray(np.asarray(b_lin, np.float32)[:, None])

    in_maps = []
    for c in range(P):
        ef = percd[c]
        er = percd[P + c]

        # gather idx [128, EHP//16], hop-1 (natural table cols) and hop-2
        # (rank-composed cols of the AllGather'ed tables)
        gidx = np.empty((128, EHP // 16), np.int16)
        gidx2 = np.empty((128, EHP // 16), np.int16)
        for d, e in ((0, ef), (1, er)):
            slots = off[e["j"]] + e["rank"][e["tw"]]
            vals1 = e["sw"] - e["ww"] * HALF
            cs = e["sw"] // NLOC
            ls = e["sw"] - cs * NLOC
            rk = np.empty(len(cs), np.int64)
            for csv in range(P):
                msk = cs == csv
                rk[msk] = percd[d * P + csv]["rank"][ls[msk]]
            vals2 = (cs % 4) * NLOC + rk
            for sub in range(2):
                for wv in range(2):
                    msk = (e["sub"] == sub) & (e["ww"] == wv)
                    k = _slab(sub, wv, d)
                    iv = np.full(EHP, PADIDX, np.int64)
                    iv[slots[msk]] = vals1[msk]
                    gidx[16 * k:16 * k + 16] = _wrap16(iv)
                    iv2 = np.full(EHP, PADIDX, np.int64)
                    iv2[slots[msk]] = vals2[msk]
                    gidx2[16 * k:16 * k + 16] = _wrap16(iv2)

        # perm idx [128, PQ//16] (core 2q+dd handles quarter q of dir dd)
        pidx = np.empty((128, PQ // 16), np.int16)
        for q in range(4):
            for dd, e in ((0, ef), (1, er)):
                L = np.full(PQ, HP - 1, np.int64)
                qn = min(QL[q], NLOC - QS[q])
                nat = np.arange(QS[q], QS[q] + qn)
                L[:qn] = e["rank"][nat]
                k = 2 * q + dd
                pidx[16 * k:16 * k + 16] = _wrap16(L)

        # dscale [128, 2*HP] fp16 (rank order, quadruplicated)
        dsc = np.zeros((32, HP), np.float32)
        gl = c * NLOC
        dsc[0:16, :NLOC] = doi1[gl + ef["ordm"]][None, :]
        dsc[16:32, :NLOC] = dii1[gl + er["ordm"]][None, :]
        dsc = np.repeat(dsc[:, :, None], 2, axis=2).reshape(32, 2 * HP)
        dsc = np.tile(dsc, (4, 1)).astype(F16NP)

        # recovery [128, 2*PQ] f32 (natural order, quarter layout)
        rcv = np.zeros((128, 2 * PQ), np.float32)
        for q in range(4):
            qn = min(QL[q], NLOC - QS[q])
            nat = gl + QS[q] + np.arange(qn)
            rcv[32 * q:32 * q + 16, :2 * qn] = np.repeat(dgo1[nat], 2)[None]
            rcv[32 * q + 16:32 * q + 32, :2 * qn] = \
                np.repeat(dgi1[nat], 2)[None]

        # xT [32, HP] fp16 natural feature-major
        xT = np.zeros((F, HP), np.float32)
        xT[:, :NLOC] = x[gl:gl + NLOC].T
        xT = xT.astype(F16NP)

        in_maps.append({
            "tab1": tab1, "gidx": gidx, "gidx2": gidx2, "pidx": pidx,
            "dsc": np.ascontiguousarray(dsc),
            "rcv": np.ascontiguousarray(rcv),
            "xT": np.ascontiguousarray(xT),
            "selw": selw.astype(F16NP), "wq": WQ, "wx": WX,
            "nbz": nbz, "bh": bhv, "wl": WL, "bl": BL,
        })

    meta = dict(NJ=[int(v) for v in NJ], off=[int(v) for v in off],
                EH=EH, EHP=EHP, chunk=chunk)
    return in_maps, meta


# ----------------------------------------------------------------------------
# Device program
# ----------------------------------------------------------------------------

def build_program(meta):
    NJ, off, EHP = meta["NJ"], meta["off"], meta["EHP"]
    CHUNK = meta["chunk"]
    nrounds = len(NJ)
    NSEL = _ceil(2 * HP, 512)
    gate_chunks = []
    n0 = 0
    while n0 < HP:
        n1 = min(n0 + NT, HP)
        q = 3
        for qq in range(4):
            if n0 < QS[qq] + QL[qq]:
                q = qq
                break
        gate_chunks.append((n0, n1, q))
        n0 = n1

    nc = bacc.Bacc("TRN2", target_bir_lowering=False, debug=False,
                   num_devices=P)

    def din(name, shape, dt=FP32):
        return nc.dram_tensor(name, list(shape), dt, kind="ExternalInput").ap()

    tab1_d = din("tab1", (128, 2 * NE), FP16)
    gidx_d = din("gidx", (128, EHP // 16), I16)
    gidx2_d = din("gidx2", (128, EHP // 16), I16)
    pidx_d = din("pidx", (128, PQ // 16), I16)
    dsc_d = din("dsc", (128, 2 * HP), FP16)
    rcv_d = din("rcv", (128, 2 * PQ), FP32)
    xT_d = din("xT", (F, HP), FP16)
    selw_d = din("selw", (128, 128), FP16)
    wq_d = din("wq", (128, 16 * 128), FP16)
    wx_d = din("wx", (F, 128), FP16)
    nbz_d = din("nbz", (FO, 1), FP32)
    bh_d = din("bh", (FO, 1), FP32)
    wl_d = din("wl", (FO, PER), FP32)
    bl_d = din("bl", (PER, 1), FP32)
    out_d = nc.dram_tensor("out", [PER, HP], FP32, kind="ExternalOutput").ap()

    with tile.TileContext(nc) as tc:
        from contextlib import ExitStack
        with ExitStack() as ctx:
            sb = ctx.enter_context(tc.tile_pool(name="sb", bufs=1))
            gp = ctx.enter_context(tc.tile_pool(name="gp", bufs=2))
            dp = ctx.enter_context(tc.tile_pool(name="dp", bufs=2))
            xp = ctx.enter_context(tc.tile_pool(name="xp", bufs=2))
            op = ctx.enter_context(tc.tile_pool(name="op", bufs=2))
            ps = ctx.enter_context(tc.tile_pool(name="ps", bufs=2,
                                                space="PSUM"))
            pg = ctx.enter_context(tc.tile_pool(name="pg", bufs=2,
                                                space="PSUM"))
            ph = ctx.enter_context(tc.tile_pool(name="ph", bufs=2,
                                                space="PSUM"))
            dr = ctx.enter_context(tc.tile_pool(name="dr", bufs=1,
                                                space="DRAM"))

            tab = sb.tile([128, 2 * NE], FP16, tag="tab")
            gidx = sb.tile([128, EHP // 16], I16, tag="gidx")
            pidx = sb.tile([128, PQ // 16], I16, tag="pidx")
            rcv = sb.tile([128, 2 * PQ], FP32, tag="rcv")
            selw = sb.tile([128, 128], FP16, tag="selw")
            wq = sb.tile([128, 16 * 128], FP16, tag="wq")
            wx = sb.tile([F, 128], FP16, tag="wx")
            nbz = sb.tile([FO, 1], FP32, tag="nbz")
            bh = sb.tile([FO, 1], FP32, tag="bh")
            wl = sb.tile([FO, PER], FP32, tag="wl")
            bl = sb.tile([PER, 1], FP32, tag="bl")
            acc = sb.tile([128, 2 * HP], FP16, tag="acc")
            t1n = sb.tile([128, 2 * PQ], FP16, tag="t1n")

            cT = dr.tile([32, 2 * NLOC], FP16, tag="cT")
            agT = dr.tile([256, 2 * NLOC], FP16, tag="agT")

            nc.sync.dma_start(tab[:], tab1_d)
            nc.sync.dma_start(gidx[:], gidx_d)
            nc.sync.dma_start(pidx[:], pidx_d)
            nc.sync.dma_start(rcv[:], rcv_d)
            nc.sync.dma_start(selw[:], selw_d)
            nc.sync.dma_start(wq[:], wq_d)
            nc.sync.dma_start(wx[:], wx_d)
            nc.sync.dma_start(nbz[:], nbz_d)
            nc.sync.dma_start(bh[:], bh_d)
            nc.sync.dma_start(wl[:], wl_d)
            nc.sync.dma_start(bl[:], bl_d)
            nc.vector.memset(acc[:], 0.0)

            tabv = tab[:].rearrange("p (n d) -> p n d", d=2)

            def hop():
                for cc in range(NCALLS):
                    g = gp.tile([128, CHUNK, 2], FP16, tag="g")
                    nc.gpsimd.ap_gather(
                        out_ap=g[:],
                        in_ap=tabv,
                        idxs_ap=gidx[:, cc * (CHUNK // 16):
                                     (cc + 1) * (CHUNK // 16)],
                        channels=128,
                        num_elems=NE,
                        d=2,
                        num_idxs=CHUNK,
                    )
                    gv = g[:].rearrange("p n d -> p (n d)")
                    c0, c1 = cc * CHUNK, (cc + 1) * CHUNK
                    for j in range(nrounds):
                        s = max(off[j], c0)
                        e = min(off[j] + NJ[j], c1)
                        if s >= e:
                            continue
                        r0 = s - off[j]
                        nc.vector.tensor_tensor(
                            out=acc[:, 2 * r0:2 * (r0 + e - s)],
                            in0=acc[:, 2 * r0:2 * (r0 + e - s)],
                            in1=gv[:, 2 * (s - c0):2 * (e - c0)],
                            op=mybir.AluOpType.add,
                        )

            def selpass(scale):
                """sel-matmul acc -> fp16 (scaled if scale) into tab[:,
                0:2*HP] (tab is dead at both call sites)."""
                accb = tab[:, 0:2 * HP]
                for si in reversed(range(NSEL)):
                    s0 = si * 512
                    s1 = min(s0 + 512, 2 * HP)
                    pt = ps.tile([128, 512], FP32, tag="sel")
                    nc.tensor.matmul(out=pt[:, :s1 - s0], lhsT=selw[:],
                                     rhs=acc[:, s0:s1], start=True, stop=True)
                    if scale:
                        dt_ = dp.tile([128, 512], FP16, tag="dsc")
                        nc.sync.dma_start(dt_[:, :s1 - s0], dsc_d[:, s0:s1])
                        nc.vector.tensor_tensor(
                            out=accb[:, s0:s1], in0=pt[:, :s1 - s0],
                            in1=dt_[:, :s1 - s0], op=mybir.AluOpType.mult)
                    else:
                        nc.vector.tensor_copy(out=accb[:, s0:s1],
                                              in_=pt[:, :s1 - s0])
                return accb

            def permute(src, dst):
                nc.gpsimd.ap_gather(
                    out_ap=dst.rearrange("p (n d) -> p n d", d=2),
                    in_ap=src.rearrange("p (n d) -> p n d", d=2),
                    idxs_ap=pidx[:],
                    channels=128,
                    num_elems=HP,
                    d=2,
                    num_idxs=PQ,
                )

            # ---- hop 1 -----------------------------------------------------
            hop()
            accb1 = selpass(scale=True)
            # re-zero acc early (overlaps collective/loads)
            nc.vector.memset(acc[:], 0.0)
            # contribution in RANK order straight from the sel output
            nc.scalar.dma_start(cT[:], accb1[0:32, 0:2 * NLOC])
            nc.gpsimd.collective_compute(
                "AllGather", mybir.AluOpType.bypass,
                replica_groups=[list(range(P))],
                ins=[cT[:].opt()],
                outs=[agT[:].opt()],
            )
            # natural-order T1 for the gates: permute + recovery, runs on
            # Q7/DVE concurrently with the collective
            pmo = tab[:, 2 * HP:2 * HP + 2 * PQ]
            permute(accb1, pmo)
            nc.vector.tensor_tensor(out=t1n[:], in0=pmo[:, :], in1=rcv[:],
                                    op=mybir.AluOpType.mult)

            # ---- hop-2 table load: 4 wide 32-partition DMAs ---------------
            agv = agT[:].rearrange("(w c2 r) c -> w r c2 c", w=2, c2=4, r=32)
            for cp, w in ((0, 0), (0, 1), (1, 0), (1, 1)):
                eng = nc.sync if cp == w else nc.scalar
                p0 = 64 * cp + 32 * w
                eng.dma_start(
                    tab[p0:p0 + 32, 0:2 * 4 * NLOC]
                    .rearrange("p (c2 c) -> p c2 c", c2=4),
                    agv[w])
            # swap in hop-2 idx lists (after the table loads on the sync
            # ring so they aren't delayed; needed only by hop-2's gathers)
            nc.sync.dma_start(gidx[:], gidx2_d)

            # ---- hop 2 -----------------------------------------------------
            hop()
            accb2 = selpass(scale=False)
            y2n = tab[:, 2 * HP:2 * HP + 2 * PQ]
            permute(accb2, y2n)

            # ---- gates + head ---------------------------------------------
            t1v = t1n[:].rearrange("p (n d) -> p n d", d=2)
            y2v = y2n.rearrange("p (n d) -> p n d", d=2)
            for (n0, n1, q) in gate_chunks:
                n = n1 - n0
                lc = n0 - QS[q]
                xt = xp.tile([F, NT], FP16, tag="xt")
                nc.sync.dma_start(xt[:, :n], xT_d[:, n0:n1])
                pz = pg.tile([FO, NT], FP32, tag="pz")
                phh = pg.tile([FO, NT], FP32, tag="ph")
                for half, pt in ((0, pz), (1, phh)):
                    co = FO * half
                    first = True
                    for term, vv in ((0, y2v), (1, t1v)):
                        for hh in range(2):
                            b = (term * 2 + hh) * 4 + q
                            nc.tensor.matmul(
                                out=pt[:, :n],
                                lhsT=wq[:, 128 * b + co:128 * b + co + FO],
                                rhs=vv[:, lc:lc + n, hh],
                                start=first, stop=False)
                            first = False
                    nc.tensor.matmul(out=pt[:, :n],
                                     lhsT=wx[:, co:co + FO],
                                     rhs=xt[:, :n], start=False, stop=True)
                AF = mybir.ActivationFunctionType
                gz = op.tile([FO, NT], FP32, tag="gz")
                gh = op.tile([FO, NT], FP32, tag="gh")
                nc.scalar.activation(out=gz[:, :n], in_=pz[:, :n],
                                     func=AF.Sigmoid, bias=nbz[:], scale=-1.0)
                nc.scalar.activation(out=gh[:, :n], in_=phh[:, :n],
                                     func=AF.Tanh, bias=bh[:], scale=1.0)
                nc.vector.tensor_tensor(out=gz[:, :n], in0=gz[:, :n],
                                        in1=gh[:, :n],
                                        op=mybir.AluOpType.mult)
                nc.vector.tensor_scalar_max(gz[:, :n], gz[:, :n], 0.0)
                po = ph.tile([PER, NT], FP32, tag="po")
                nc.tensor.matmul(out=po[:, :n], lhsT=wl[:], rhs=gz[:, :n],
                                 start=True, stop=True)
                ot = op.tile([PER, NT], FP32, tag="ot")
                nc.scalar.add(out=ot[:, :n], in_=po[:, :n], add=bl[:])
                nc.scalar.dma_start(out_d[:, n0:n1], ot[:, :n])

    nc.compile()
    return nc


# ----------------------------------------------------------------------------
# Entry point
# ----------------------------------------------------------------------------

def kernel(x, edge_index, edge_weight, w_z, b_z, w_r, b_r, w_h, b_h,
           w_lin, b_lin, _trace=False):
    in_maps, meta = preprocess(x, edge_index, edge_weight, w_z, b_z, w_r,
                               b_r, w_h, b_h, w_lin, b_lin)
    nc = build_program(meta)
    res = bass_utils.run_bass_kernel_spmd(
        nc, in_maps, core_ids=list(range(P)), trace=_trace)
    out = np.empty((N, PER), np.float32)
    for c in range(P):
        out[c * NLOC:(c + 1) * NLOC] = res.results[c]["out"].T[:NLOC]
    if _trace:
        return out, res
    return out
